# revision 3
# baseline (speedup 1.0000x reference)
"""CRF loss kernel for Trainium2 (8 NeuronCores, data-parallel over batch).

Strategy (v2): the CRF forward recurrence is a chain of 512 per-step 32x32
positive matrices in the exp domain:  w_final = (M_{S-1}/c)...(M_0/c) @ 1.
Instead of a serial scan, the kernel computes 4-step partial products with a
two-level parallel product tree on the TensorEngine (4 examples packed
block-diagonally per 128x128 stationary), ships the 4-step products (bf16)
to the host, and the host finishes the (trivial) 128-step vector chain in
float64.  Padded timesteps (t >= len_b) are masked host-side to exact
identity matrices (diag = log c, off-diag = -30), so no per-step traj/renorm
bookkeeping is needed on device.  The gold score is a tiny host-side gather.

Per core: 8 examples; groups g in {0,1} of 4 examples q in {0..3}; example
b_local = g*4 + q.  Partition layout everywhere: p = (q, tag).

Orientation convention: "N-form" = [row-on-partition, col-on-free];
"T-form" = [col-on-partition, row-on-free].  A pair product C_k =
A_{2k+1} @ A_{2k} computed as matmul(lhsT=blockdiag(stationary), rhs=moving)
yields N-form when the stationary is the odd (T-form) child, and T-form when
the stationary is the even (N-form) child.  Level-0 alternates N/T by pair
parity so level-1 gets its stationaries (odd slabs, T-form) for free; level-1
products are all produced N-form for the host.
"""

import numpy as np

B, S, T = 64, 512, 32
NCORES = 8
BPC = B // NCORES          # examples per core
QG, G = 4, 2               # partition-block examples, groups
TC = 32                    # timesteps per chunk
NCH = S // TC              # chunks
NPAIR = TC // 2            # level-0 pairs per group per chunk
NM = TC // 4               # level-1 (4-step) products per group per chunk
LOG_C = 4.0                # fixed per-step scale (log domain)
NEG = -30.0                # off-diagonal of identity-in-log-domain pattern
END = T - 1

_CACHE = {}


def _build():
    import concourse.bass as bass
    import concourse.tile as tile
    from concourse import bacc, mybir

    f32 = mybir.dt.float32
    bf16 = mybir.dt.bfloat16
    AF = mybir.ActivationFunctionType

    nc = bacc.Bacc("TRN2", target_bir_lowering=False, debug=False,
                   enable_asserts=False)

    sc = nc.dram_tensor("sc", [128, S * G * T], f32, kind="ExternalInput").ap()
    outp = nc.dram_tensor("outp", [128, NCH * NM * G * T], bf16,
                          kind="ExternalOutput").ap()

    with tile.TileContext(nc) as tc:
        with (
            tc.tile_pool(name="pin", bufs=2) as pin,
            tc.tile_pool(name="pcmp", bufs=2) as pcmp,
            tc.tile_pool(name="ptodd", bufs=2) as ptodd,
            tc.tile_pool(name="pbd", bufs=1) as pbd,
            tc.tile_pool(name="pm1", bufs=2) as pm1,
            tc.tile_pool(name="pout", bufs=2) as pout,
            tc.tile_pool(name="pp1", bufs=2, space="PSUM") as pp1,
            tc.tile_pool(name="pp2", bufs=2, space="PSUM") as pp2,
        ):
            bd0 = [pbd.tile([128, 2 * NPAIR * 128], bf16, tag=f"bd0_{i}",
                            name=f"bd0_{i}") for i in range(2)]
            bd1 = [pbd.tile([128, NPAIR * 128], bf16, tag=f"bd1_{i}",
                            name=f"bd1_{i}") for i in range(2)]
            for t_ in bd0 + bd1:
                nc.vector.memset(t_[:], 0.0)
            bias_t = pbd.tile([128, 1], f32, tag="bias", name="bias_t")
            nc.vector.memset(bias_t[:], -LOG_C)

            CW = TC * G * T  # chunk width in input cols (2048)
            for c in range(NCH):
                stg = pin.tile([128, CW], f32, tag="stg")
                nc.sync.dma_start(stg[:], sc[:, c * CW:(c + 1) * CW])
                cmp = pcmp.tile([128, CW], bf16, tag="cmp")
                nc.scalar.activation(cmp[:], stg[:], AF.Exp, bias=bias_t[:])

                # T-form of odd-t matrices via 32x32 block transpose.
                # cmp layout per chunk: (parity e, tau, g, j) — odd-t half is
                # the contiguous second half, so src/dst are flat 2D.
                todd = ptodd.tile([128, NPAIR * G * T], bf16, tag="todd")
                nc.vector.transpose(todd[:], cmp[:, CW // 2:CW])

                B0, B1 = bd0[c % 2], bd1[c % 2]
                # level-0 stationary scatter into block-diag slots s = 2k+g
                b0v = B0[:].rearrange("p (m r x) -> p m r x", m=NPAIR // 2,
                                      r=4, x=128)
                tv = todd[:].rearrange("p (m e g j) -> p m e g j",
                                       m=NPAIR // 2, e=2, g=G, j=T)
                cev = cmp[:, :CW // 2].rearrange("p (m o g j) -> p m o g j",
                                                 m=NPAIR // 2, o=2, g=G, j=T)
                for q in range(QG):
                    ps, pe = 32 * q, 32 * q + 32
                    # k even: stationary = A_{2k+1} (T-form), slots r in {0,1}
                    nc.vector.tensor_copy(b0v[ps:pe, :, 0:2, ps:pe],
                                          tv[ps:pe, :, 0, :, :])
                    # k odd: stationary = A_{2k} (N-form, t=4m+2), slots {2,3}
                    nc.vector.tensor_copy(b0v[ps:pe, :, 2:4, ps:pe],
                                          cev[ps:pe, :, 1, :, :])

                # level-0 pair products: C_k (N-form if k even, T-form if odd)
                P1 = pp1.tile([128, NPAIR * G * T], f32, tag="p1")
                for k in range(NPAIR):
                    for g in range(G):
                        lhsT = B0[:, (2 * k + g) * 128:(2 * k + g + 1) * 128]
                        if k % 2 == 0:
                            # A_{2k}: even-t half, tau = k
                            rhs = cmp[:, (k * G + g) * T:(k * G + g + 1) * T]
                        else:
                            # A_{2k+1}: odd-t (T-form), tau = k
                            rhs = todd[:, (k * G + g) * T:(k * G + g + 1) * T]
                        sl = (k * G + g) * T
                        nc.tensor.matmul(
                            P1[:, sl:sl + T], lhsT, rhs,
                            start=((k, g) in ((0, 0), (8, 0))),
                            stop=((k, g) in ((7, 1), (15, 1))))

                # level-1 stationaries: odd slabs of P1 (T-form) -> B1
                p1v = P1[:].rearrange("p (m o g j) -> p m o g j", m=NM, o=2,
                                      g=G, j=T)
                b1v = B1[:].rearrange("p (m s x) -> p m s x", m=NM, s=2, x=128)
                for q in range(QG):
                    ps, pe = 32 * q, 32 * q + 32
                    nc.scalar.copy(b1v[ps:pe, :, :, ps:pe],
                                   p1v[ps:pe, :, 1, :, :])
                # level-1 movings: even slabs of P1 (N-form) -> M1 (bf16)
                M1 = pm1.tile([128, NM * G * T], bf16, tag="m1")
                nc.scalar.copy(M1[:].rearrange("p (m g j) -> p m g j", m=NM,
                                               g=G, j=T),
                               p1v[:, :, 0, :, :])

                # level-1 products: D_m = C_{2m+1} @ C_{2m}, all N-form
                P2 = pp2.tile([128, NM * G * T], f32, tag="p2")
                for m in range(NM):
                    for g in range(G):
                        lhsT = B1[:, (2 * m + g) * 128:(2 * m + g + 1) * 128]
                        sl = (m * G + g) * T
                        nc.tensor.matmul(
                            P2[:, sl:sl + T], lhsT, M1[:, sl:sl + T],
                            start=((m, g) == (0, 0)),
                            stop=((m, g) == (NM - 1, G - 1)))

                ost = pout.tile([128, NM * G * T], bf16, tag="ost")
                nc.vector.tensor_copy(ost[:], P2[:])
                nc.sync.dma_start(
                    outp[:, c * NM * G * T:(c + 1) * NM * G * T], ost[:])

    nc.compile()
    return nc


_IDPAT = np.full((T, T), NEG, np.float32)
np.fill_diagonal(_IDPAT, LOG_C)


def _prep_core_inputs(scores_core, lengths_core):
    """Mask padded steps to identity-in-log-domain, relayout to device form."""
    buf = scores_core.copy()
    for b in range(BPC):
        ln = int(lengths_core[b])
        if ln < S:
            buf[b, ln:] = _IDPAT
    # device chunk layout: (c, parity e, tau, g, j); global t = c*TC + 2*tau + e
    dev = buf.reshape(G, QG, NCH, TC // 2, 2, T, T)  # [g, q, c, tau, e, i, j]
    dev = np.transpose(dev, (1, 5, 2, 4, 3, 0, 6))   # [q, i, c, e, tau, g, j]
    return {"sc": np.ascontiguousarray(dev).reshape(128, S * G * T)}


def _postprocess(results, lengths, gold):
    """Chain the 4-step products per example in float64, add scale correction."""
    NP = NCH * NM                                 # products per example (128)
    D = np.empty((B, NP, T, T), np.float64)
    for core in range(NCORES):
        o = np.asarray(results[core]["outp"]).astype(np.float64)
        o = o.reshape(QG, T, NCH, NM, G, T)       # [q, a, c, m, g, b]
        for blc in range(BPC):
            g, q = blc // QG, blc % QG
            D[core * BPC + blc] = np.transpose(o[q, :, :, :, g, :],
                                               (1, 2, 0, 3)).reshape(NP, T, T)
    w = np.ones((B, T), np.float64)
    for p in range(NP):
        w = np.einsum('eab,eb->ea', D[:, p], w)
    total = float(np.sum(np.log(w[:, END]) + lengths.astype(np.float64) * LOG_C))
    return np.float32(total - gold)


def _gold(scores, targets, lengths):
    flat = scores.reshape(B, S, T * T)
    g = np.take_along_axis(flat, targets.astype(np.int64)[..., None],
                           axis=2)[..., 0]
    mask = np.arange(S)[None, :] < lengths[:, None]
    return float(np.sum(np.where(mask, g.astype(np.float64), 0.0)))


def kernel(scores, targets, lengths):
    from concourse import bass_utils

    scores = np.asarray(scores)
    targets = np.asarray(targets)
    lengths = np.asarray(lengths)

    if "nc" not in _CACHE:
        _CACHE["nc"] = _build()
    nc = _CACHE["nc"]

    gold = _gold(scores, targets, lengths)
    in_maps = []
    for core in range(NCORES):
        sl = slice(core * BPC, (core + 1) * BPC)
        in_maps.append(_prep_core_inputs(scores[sl], lengths[sl]))

    res = bass_utils.run_bass_kernel_spmd(nc, in_maps,
                                          core_ids=list(range(NCORES)))
    _CACHE["last_results"] = res.results
    _CACHE["last_res"] = res
    return _postprocess(res.results, lengths, gold)


# revision 4
# speedup vs baseline: 5.5818x; 5.5818x over previous
"""CRF loss kernel for Trainium2 (8 NeuronCores, data-parallel over batch).

Strategy (v2): the CRF forward recurrence is a chain of 512 per-step 32x32
positive matrices in the exp domain:  w_final = (M_{S-1}/c)...(M_0/c) @ 1.
Instead of a serial scan, the kernel computes 4-step partial products with a
two-level parallel product tree on the TensorEngine (4 examples packed
block-diagonally per 128x128 stationary), ships the 4-step products (bf16)
to the host, and the host finishes the (trivial) 128-step vector chain in
float64.  Padded timesteps (t >= len_b) are masked host-side to exact
identity matrices (diag = log c, off-diag = -30), so no per-step traj/renorm
bookkeeping is needed on device.  The gold score is a tiny host-side gather.

Per core: 8 examples; groups g in {0,1} of 4 examples q in {0..3}; example
b_local = g*4 + q.  Partition layout everywhere: p = (q, tag).

Orientation convention: "N-form" = [row-on-partition, col-on-free];
"T-form" = [col-on-partition, row-on-free].  A pair product C_k =
A_{2k+1} @ A_{2k} computed as matmul(lhsT=blockdiag(stationary), rhs=moving)
yields N-form when the stationary is the odd (T-form) child, and T-form when
the stationary is the even (N-form) child.  Level-0 alternates N/T by pair
parity so level-1 gets its stationaries (odd slabs, T-form) for free; level-1
products are all produced N-form for the host.
"""

import numpy as np

B, S, T = 64, 512, 32
NCORES = 8
BPC = B // NCORES          # examples per core
QG, G = 4, 2               # partition-block examples, groups
TC = 32                    # timesteps per chunk
NCH = S // TC              # chunks
NPAIR = TC // 2            # level-0 pairs per group per chunk
NM = TC // 4               # level-1 (4-step) products per group per chunk
LOG_C = 4.0                # fixed per-step scale (log domain)
NEG = -30.0                # off-diagonal of identity-in-log-domain pattern
END = T - 1

_CACHE = {}


def _build():
    import concourse.bass as bass
    import concourse.tile as tile
    from concourse import bacc, mybir

    f32 = mybir.dt.float32
    bf16 = mybir.dt.bfloat16
    AF = mybir.ActivationFunctionType

    nc = bacc.Bacc("TRN2", target_bir_lowering=False, debug=False,
                   enable_asserts=False)

    sc = nc.dram_tensor("sc", [128, S * G * T], bf16,
                        kind="ExternalInput").ap()
    outp = nc.dram_tensor("outp", [128, NCH * NM * G * T], bf16,
                          kind="ExternalOutput").ap()

    with tile.TileContext(nc) as tc:
        with (
            tc.tile_pool(name="pin", bufs=2) as pin,
            tc.tile_pool(name="pcmp", bufs=2) as pcmp,
            tc.tile_pool(name="ptodd", bufs=2) as ptodd,
            tc.tile_pool(name="pbd", bufs=1) as pbd,
            tc.tile_pool(name="pm1", bufs=2) as pm1,
            tc.tile_pool(name="pout", bufs=2) as pout,
            tc.tile_pool(name="pp1", bufs=2, space="PSUM") as pp1,
            tc.tile_pool(name="pp2", bufs=2, space="PSUM") as pp2,
        ):
            bd0 = [pbd.tile([128, 2 * NPAIR * 128], bf16, tag=f"bd0_{i}",
                            name=f"bd0_{i}") for i in range(2)]
            bd1 = [pbd.tile([128, NPAIR * 128], bf16, tag=f"bd1_{i}",
                            name=f"bd1_{i}") for i in range(2)]
            for t_ in bd0 + bd1:
                nc.vector.memset(t_[:], 0.0)
            bias_t = pbd.tile([128, 1], f32, tag="bias", name="bias_t")
            nc.vector.memset(bias_t[:], -LOG_C)

            CW = TC * G * T  # chunk width in input cols (2048)
            for c in range(NCH):
                stg = pin.tile([128, CW], bf16, tag="stg")
                nc.sync.dma_start(stg[:], sc[:, c * CW:(c + 1) * CW])
                cmp = pcmp.tile([128, CW], bf16, tag="cmp")
                nc.scalar.activation(cmp[:], stg[:], AF.Exp, bias=bias_t[:])

                # T-form of odd-t matrices via 32x32 block transpose.
                # cmp layout per chunk: (parity e, tau, g, j) — odd-t half is
                # the contiguous second half, so src/dst are flat 2D.
                todd = ptodd.tile([128, NPAIR * G * T], bf16, tag="todd")
                nc.vector.transpose(todd[:], cmp[:, CW // 2:CW])

                B0, B1 = bd0[c % 2], bd1[c % 2]
                # level-0 stationary scatter into block-diag slots s = 2k+g
                b0v = B0[:].rearrange("p (m r x) -> p m r x", m=NPAIR // 2,
                                      r=4, x=128)
                tv = todd[:].rearrange("p (m e g j) -> p m e g j",
                                       m=NPAIR // 2, e=2, g=G, j=T)
                cev = cmp[:, :CW // 2].rearrange("p (m o g j) -> p m o g j",
                                                 m=NPAIR // 2, o=2, g=G, j=T)
                for q in range(QG):
                    ps, pe = 32 * q, 32 * q + 32
                    # k even: stationary = A_{2k+1} (T-form), slots r in {0,1}
                    nc.vector.tensor_copy(b0v[ps:pe, :, 0:2, ps:pe],
                                          tv[ps:pe, :, 0, :, :])
                    # k odd: stationary = A_{2k} (N-form, t=4m+2), slots {2,3}
                    nc.vector.tensor_copy(b0v[ps:pe, :, 2:4, ps:pe],
                                          cev[ps:pe, :, 1, :, :])

                # level-0 pair products: C_k (N-form if k even, T-form if odd)
                P1 = pp1.tile([128, NPAIR * G * T], f32, tag="p1")
                for k in range(NPAIR):
                    for g in range(G):
                        lhsT = B0[:, (2 * k + g) * 128:(2 * k + g + 1) * 128]
                        if k % 2 == 0:
                            # A_{2k}: even-t half, tau = k
                            rhs = cmp[:, (k * G + g) * T:(k * G + g + 1) * T]
                        else:
                            # A_{2k+1}: odd-t (T-form), tau = k
                            rhs = todd[:, (k * G + g) * T:(k * G + g + 1) * T]
                        sl = (k * G + g) * T
                        nc.tensor.matmul(
                            P1[:, sl:sl + T], lhsT, rhs,
                            start=((k, g) in ((0, 0), (8, 0))),
                            stop=((k, g) in ((7, 1), (15, 1))))

                # level-1 stationaries: odd slabs of P1 (T-form) -> B1
                p1v = P1[:].rearrange("p (m o g j) -> p m o g j", m=NM, o=2,
                                      g=G, j=T)
                b1v = B1[:].rearrange("p (m s x) -> p m s x", m=NM, s=2, x=128)
                for q in range(QG):
                    ps, pe = 32 * q, 32 * q + 32
                    nc.scalar.copy(b1v[ps:pe, :, :, ps:pe],
                                   p1v[ps:pe, :, 1, :, :])
                # level-1 movings: even slabs of P1 (N-form) -> M1 (bf16)
                M1 = pm1.tile([128, NM * G * T], bf16, tag="m1")
                nc.scalar.copy(M1[:].rearrange("p (m g j) -> p m g j", m=NM,
                                               g=G, j=T),
                               p1v[:, :, 0, :, :])

                # level-1 products: D_m = C_{2m+1} @ C_{2m}, all N-form
                P2 = pp2.tile([128, NM * G * T], f32, tag="p2")
                for m in range(NM):
                    for g in range(G):
                        lhsT = B1[:, (2 * m + g) * 128:(2 * m + g + 1) * 128]
                        sl = (m * G + g) * T
                        nc.tensor.matmul(
                            P2[:, sl:sl + T], lhsT, M1[:, sl:sl + T],
                            start=((m, g) == (0, 0)),
                            stop=((m, g) == (NM - 1, G - 1)))

                ost = pout.tile([128, NM * G * T], bf16, tag="ost")
                nc.vector.tensor_copy(ost[:], P2[:])
                nc.sync.dma_start(
                    outp[:, c * NM * G * T:(c + 1) * NM * G * T], ost[:])

    nc.compile()
    return nc


def _bf16():
    import ml_dtypes
    return ml_dtypes.bfloat16


_IDPAT = np.full((T, T), NEG, np.float32)
np.fill_diagonal(_IDPAT, LOG_C)


def _prep_core_inputs(scores_core, lengths_core):
    """Mask padded steps to identity-in-log-domain, relayout to device form,
    cast to bf16 (halves transfer + device DMA; rel err stays ~1e-5)."""
    bf16 = _bf16()
    buf = scores_core.astype(bf16)
    idp = _IDPAT.astype(bf16)
    for b in range(BPC):
        ln = int(lengths_core[b])
        if ln < S:
            buf[b, ln:] = idp
    # device chunk layout: (c, parity e, tau, g, j); global t = c*TC + 2*tau + e
    dev = buf.reshape(G, QG, NCH, TC // 2, 2, T, T)  # [g, q, c, tau, e, i, j]
    dev = np.transpose(dev, (1, 5, 2, 4, 3, 0, 6))   # [q, i, c, e, tau, g, j]
    return {"sc": np.ascontiguousarray(dev).reshape(128, S * G * T)}


def _postprocess(results, lengths, gold):
    """Chain the 4-step products per example in float64, add scale correction."""
    NP = NCH * NM                                 # products per example (128)
    D = np.empty((B, NP, T, T), np.float64)
    for core in range(NCORES):
        o = np.asarray(results[core]["outp"]).astype(np.float64)
        o = o.reshape(QG, T, NCH, NM, G, T)       # [q, a, c, m, g, b]
        for blc in range(BPC):
            g, q = blc // QG, blc % QG
            D[core * BPC + blc] = np.transpose(o[q, :, :, :, g, :],
                                               (1, 2, 0, 3)).reshape(NP, T, T)
    w = np.ones((B, T), np.float64)
    for p in range(NP):
        w = np.einsum('eab,eb->ea', D[:, p], w)
    total = float(np.sum(np.log(w[:, END]) + lengths.astype(np.float64) * LOG_C))
    return np.float32(total - gold)


def _gold(scores, targets, lengths):
    flat = scores.reshape(B, S, T * T)
    g = np.take_along_axis(flat, targets.astype(np.int64)[..., None],
                           axis=2)[..., 0]
    mask = np.arange(S)[None, :] < lengths[:, None]
    return float(np.sum(np.where(mask, g.astype(np.float64), 0.0)))


def _make_fast_runner(nc):
    """Build a cached jitted 8-core executable (same lowering
    run_bass_kernel_spmd uses under axon, but traced/compiled once)."""
    import jax
    import numpy as _np
    from jax.sharding import Mesh, PartitionSpec
    from jax.experimental.shard_map import shard_map
    from concourse import bass2jax, mybir

    bass2jax.install_neuronx_cc_hook()
    partition_name = (nc.partition_id_tensor.name
                      if nc.partition_id_tensor else None)
    in_names, out_names, out_avals, zero_shapes = [], [], [], []
    for alloc in nc.m.functions[0].allocations:
        if not isinstance(alloc, mybir.MemoryLocationSet):
            continue
        name = alloc.memorylocations[0].name
        if alloc.kind == "ExternalInput":
            if name != partition_name:
                in_names.append(name)
        elif alloc.kind == "ExternalOutput":
            out_names.append(name)
            shape = tuple(alloc.tensor_shape)
            dtype = mybir.dt.np(alloc.dtype)
            out_avals.append(jax.core.ShapedArray(shape, dtype))
            zero_shapes.append((shape, dtype))
    n_params = len(in_names)
    in_names_all = in_names + out_names + (
        [partition_name] if partition_name else [])

    def _body(*args):
        operands = list(args)
        if partition_name:
            operands.append(bass2jax.partition_id_tensor())
        return tuple(bass2jax._bass_exec_p.bind(
            *operands, out_avals=tuple(out_avals),
            in_names=tuple(in_names_all), out_names=tuple(out_names),
            lowering_input_output_aliases=(), sim_require_finite=True,
            sim_require_nnan=True, nc=nc))

    devices = jax.devices()[:NCORES]
    mesh = Mesh(_np.asarray(devices), ("core",))
    n_outs = len(out_avals)
    sharded = jax.jit(
        shard_map(_body, mesh=mesh,
                  in_specs=(PartitionSpec("core"),) * (n_params + n_outs),
                  out_specs=(PartitionSpec("core"),) * n_outs,
                  check_rep=False),
        donate_argnums=tuple(range(n_params, n_params + n_outs)),
        keep_unused=True)

    def run(in_maps):
        concat_in = [
            _np.concatenate([_np.asarray(m[n]) for m in in_maps], axis=0)
            for n in in_names]
        zeros = [_np.zeros((NCORES * s[0], *s[1:]), d)
                 for s, d in zero_shapes]
        outs = sharded(*concat_in, *zeros)
        return [
            {name: _np.asarray(outs[i]).reshape(NCORES, *out_avals[i].shape)[c]
             for i, name in enumerate(out_names)}
            for c in range(NCORES)]

    return run


def kernel(scores, targets, lengths):
    from concourse import bass_utils

    scores = np.asarray(scores)
    targets = np.asarray(targets)
    lengths = np.asarray(lengths)

    if "nc" not in _CACHE:
        _CACHE["nc"] = _build()
    nc = _CACHE["nc"]

    gold = _gold(scores, targets, lengths)
    in_maps = []
    for core in range(NCORES):
        sl = slice(core * BPC, (core + 1) * BPC)
        in_maps.append(_prep_core_inputs(scores[sl], lengths[sl]))

    results = None
    if "runner" in _CACHE:
        try:
            results = _CACHE["runner"](in_maps)
        except Exception:
            results = None
    if results is None:
        res = bass_utils.run_bass_kernel_spmd(nc, in_maps,
                                              core_ids=list(range(NCORES)))
        _CACHE["last_res"] = res
        results = res.results
        if "runner" not in _CACHE:
            try:
                _CACHE["runner"] = _make_fast_runner(nc)
            except Exception:
                pass
    _CACHE["last_results"] = results
    return _postprocess(results, lengths, gold)
